# revision 10
# baseline (speedup 1.0000x reference)
"""Trainium2 Bass kernel for nn_DinoText (retrieval_knn).

Computation (reference):
    t = l2norm(tanh(textual @ W.T + b))              [B, Dd]
    v = l2norm(visual, axis=-1)                      [B, P, Dd]
    sims = einsum('ik,ijk->ij', t, v); softmax; argmax -> idx  [B]
    v_best = v[b, idx[b]]                            [B, Dd]
    out = t @ v_best.T                               [B, B]

Strategy: data-parallel over batch across 8 NeuronCores (BS=128 images
each).  SBUF partition = image (BS == 128), so the per-image text row
t_norm[i] sits on partition i and the patch stream needs NO broadcast:
  - VectorE:  (v_tile * t_norm) with accum_out -> per-patch dots
  - ScalarE:  Square with accum_out            -> per-patch sq-norms
softmax is monotonic so argmax(softmax(s)) == argmax(s); the cosine
score s/sqrt(n) is compared via the monotone transform u = s*|s|/n
(division-free sign-preserving square) so no sqrt is needed.
Each core computes the output COLUMNS for its own images:
    out[:, mine] = t_all @ v_best_mine.T
so the collective is an AllGather of the (transposed) text embeddings,
issued right after the prep phase and fully hidden under the ~375us
visual-embedding stream.  The tail (argmax, indirect gather of the 128
winning patch rows, normalize, 8 transposes + 64 matmuls, output DMA)
is ~25us of serial work.
"""

import numpy as np

try:
    import concourse.bass as bass
except ImportError:  # toolchain lives in /opt in this container
    import sys

    for _p in ("/opt/pypackages", "/opt/trn_rl_repo"):
        if _p not in sys.path:
            sys.path.insert(0, _p)
    import concourse.bass as bass

import concourse.bacc as bacc
import concourse.mybir as mybir
import concourse.tile as tile
from concourse.bass_utils import run_bass_kernel_spmd
from concourse.masks import make_identity

NCORES = 8
B, P, DD, DC = 1024, 256, 1024, 512
BS = B // NCORES  # images per core (= 128 = SBUF partitions)
IPP = 2           # patch-PAIRS per DMA tile (tile = [128, IPP*2, 1024] = IPP MB)
NT = P // (2 * IPP)  # stream iterations

AF = mybir.ActivationFunctionType
ALU = mybir.AluOpType
F32 = mybir.dt.float32
BF16 = mybir.dt.bfloat16
I32 = mybir.dt.int32

KB = 2 * IPP  # patches per tile


def _build_kernel(tc, v_d, x_d, w_d, b_d, o_d):
    nc = tc.nc
    from contextlib import ExitStack

    ctx = ExitStack()
    # vload first so its SBUF zone is independent of the prep pools
    vpool = ctx.enter_context(tc.tile_pool(name="vload", bufs=6))
    const = ctx.enter_context(tc.tile_pool(name="const", bufs=1))
    persist = ctx.enter_context(tc.tile_pool(name="persist", bufs=1))
    psum_tp = ctx.enter_context(tc.tile_pool(name="pstp", bufs=3, space="PSUM"))
    psum_s = ctx.enter_context(tc.tile_pool(name="pss", bufs=1, space="PSUM"))
    psum_o = ctx.enter_context(tc.tile_pool(name="pso", bufs=2, space="PSUM"))
    dram = ctx.enter_context(tc.tile_pool(name="dram", bufs=1, space="DRAM"))

    # ---- constants -------------------------------------------------------
    ident = const.tile([128, 128], F32, tag="ident")
    make_identity(nc, ident[:])
    ones_col = const.tile([1, 128], F32, tag="ones_col")
    nc.vector.memset(ones_col[:], 1.0)
    # rowbase[i, 0] = i * P  (flat patch-row base of image i), as f32 for
    # DVE index arithmetic (exact below 2^24)
    rowbase_i = const.tile([128, 1], I32, tag="rowbase_i")
    nc.gpsimd.iota(rowbase_i[:], pattern=[[0, 1]], base=0, channel_multiplier=P)
    rowbase = const.tile([128, 1], F32, tag="rowbase")
    nc.vector.tensor_copy(rowbase[:], rowbase_i[:])

    # ---- phase 0: t_norm = l2norm(tanh(x @ W.T + b)) ---------------------
    t_norm = persist.tile([128, DD], F32, tag="t_norm")
    # tTw[q, kk, c] = t_norm[c, kk*128+q] in bf16: the AllGather payload.
    # Row q is 2KB contiguous, so the DRAM round trip uses 2KB descriptors.
    tTw = persist.tile([128, 8, 128], BF16, tag="tTw")

    with tc.tile_pool(name="prep", bufs=2) as prep, tc.tile_pool(name="wtp", bufs=1) as wtp:
        wT = [wtp.tile([128, DD], F32, tag=f"wT{j}", name=f"wT{j}") for j in range(4)]
        xT = [wtp.tile([128, 128], F32, tag=f"xT{j}", name=f"xT{j}") for j in range(4)]
        for kc in range(8):
            wn = prep.tile([128, DC], F32, tag="wn")
            nc.sync.dma_start(out=wn[:], in_=w_d[kc * 128 : (kc + 1) * 128, :])
            for j in range(4):
                pt = psum_tp.tile([128, 128], F32, tag="tp")
                nc.tensor.transpose(
                    out=pt[:], in_=wn[:, j * 128 : (j + 1) * 128], identity=ident[:]
                )
                nc.vector.tensor_copy(wT[j][:, kc * 128 : (kc + 1) * 128], pt[:])

        xn = prep.tile([128, DC], F32, tag="xn")
        nc.sync.dma_start(out=xn[:], in_=x_d[:, :])
        for j in range(4):
            pt = psum_tp.tile([128, 128], F32, tag="tp")
            nc.tensor.transpose(
                out=pt[:], in_=xn[:, j * 128 : (j + 1) * 128], identity=ident[:]
            )
            nc.vector.tensor_copy(xT[j][:], pt[:])

        bsb = const.tile([1, DD], F32, tag="bsb")
        nc.sync.dma_start(out=bsb[:], in_=b_d[:, :])

        t_sb = prep.tile([128, DD], F32, tag="t_sb")
        for h in range(2):
            tp_ps = psum_s.tile([128, 512], F32, tag="tps")
            for j in range(4):
                nc.tensor.matmul(
                    out=tp_ps[:],
                    lhsT=xT[j][:],
                    rhs=wT[j][:, h * 512 : (h + 1) * 512],
                    start=(j == 0),
                    stop=False,
                )
            nc.tensor.matmul(
                out=tp_ps[:],
                lhsT=ones_col[:],
                rhs=bsb[:, h * 512 : (h + 1) * 512],
                start=False,
                stop=True,
            )
            nc.scalar.activation(
                out=t_sb[:, h * 512 : (h + 1) * 512], in_=tp_ps[:], func=AF.Tanh
            )

        tn2 = const.tile([128, 1], F32, tag="tn2")
        tscr = prep.tile([128, DD], F32, tag="tscr")
        nc.vector.scalar_tensor_tensor(
            out=tscr[:],
            in0=t_sb[:],
            scalar=0.0,
            in1=t_sb[:],
            op0=ALU.bypass,
            op1=ALU.mult,
            accum_out=tn2[:],
        )
        tinv = const.tile([128, 1], F32, tag="tinv")
        nc.vector.reciprocal(tinv[:], tn2[:])
        trsq = const.tile([128, 1], F32, tag="trsq")
        nc.scalar.activation(out=trsq[:], in_=tinv[:], func=AF.Sqrt)
        nc.scalar.activation(out=t_norm[:], in_=t_sb[:], func=AF.Copy, scale=trsq[:])

        for kc in range(8):
            pt = psum_tp.tile([128, 128], F32, tag="tp")
            nc.tensor.transpose(
                out=pt[:], in_=t_norm[:, kc * 128 : (kc + 1) * 128], identity=ident[:]
            )
            nc.vector.tensor_copy(tTw[:, kc, :], pt[:])

    # ---- AllGather of tTw: [128, 1024] bf16 per core -> [1024, 1024] -----
    ag_in = dram.tile([128, 8 * 128], BF16, tag="agin", name="agin")
    nc.gpsimd.dma_start(out=ag_in[:], in_=tTw[:])
    import os as _os

    _ag_space = "Local" if _os.environ.get("DINO_NO_COLLECTIVE") == "1" else "Shared"
    ag_out = dram.tile(
        [NCORES * 128, 8 * 128], BF16, tag="agout", name="agout", addr_space=_ag_space
    )
    if _os.environ.get("DINO_NO_COLLECTIVE") == "1":
        for cc in range(NCORES):
            nc.gpsimd.dma_start(
                out=ag_out[cc * 128 : (cc + 1) * 128, :], in_=ag_in[:]
            )
    else:
        nc.gpsimd.collective_compute(
            "AllGather",
            ALU.bypass,
            replica_groups=[list(range(NCORES))],
            ins=[ag_in[:].opt()],
            outs=[ag_out[:].opt()],
        )

    # gathered tT in SBUF: agT[q, r, kk*128+c] = t[rank r img c, k=kk*128+q]
    agT = persist.tile([128, 8, 8 * 128], BF16, tag="agT")

    # ---- score / norm accumulators --------------------------------------
    sims = persist.tile([128, P], F32, tag="sims")
    psum_n = ctx.enter_context(tc.tile_pool(name="psn", bufs=1, space="PSUM"))
    norms = psum_n.tile([128, P], F32, tag="norms")

    dscr = ctx.enter_context(tc.tile_pool(name="dscr", bufs=1))
    nscr = ctx.enter_context(tc.tile_pool(name="nscr", bufs=1))
    gp = ctx.enter_context(tc.tile_pool(name="gp", bufs=1))

    v_flat = v_d.rearrange("b p k -> (b p) k")

    # ---- main stream: partitions = images, free = patches ----------------
    for it in range(NT):
        vt = vpool.tile([128, KB, DD], F32, tag="vt")
        nc.sync.dma_start(out=vt[:], in_=v_d[:, it * KB : (it + 1) * KB, :])
        for j in range(KB):
            p = it * KB + j
            sd = dscr.tile([128, DD], F32, tag="sd")
            nc.vector.scalar_tensor_tensor(
                out=sd[:],
                in0=vt[:, j, :],
                scalar=0.0,
                in1=t_norm[:],
                op0=ALU.bypass,
                op1=ALU.mult,
                accum_out=sims[:, p : p + 1],
            )
            sn = nscr.tile([128, DD], F32, tag="sn")
            nc.scalar.activation(
                out=sn[:],
                in_=vt[:, j, :],
                func=AF.Square,
                accum_out=norms[:, p : p + 1],
            )
        if it >= 48 and it % 2 == 0:
            # AllGather has completed by now; pull the gathered tT into
            # SBUF in 256KB chunks on the scalar HWDGE ring (a stall here
            # cannot block the v stream on the sync ring)
            r = (it - 48) // 2
            nc.scalar.dma_start(
                out=agT[:, r, :], in_=ag_out[r * 128 : (r + 1) * 128, :]
            )

    # ---- tail: argmax, gather winners, normalize, final matmul -----------
    rn = gp.tile([128, P], F32, tag="rn")
    nc.vector.reciprocal(rn[:], norms[:])
    sneg = gp.tile([128, P], F32, tag="sneg")
    nc.vector.tensor_scalar_mul(sneg[:], sims[:], -1.0)
    sabs = gp.tile([128, P], F32, tag="sabs")
    nc.vector.tensor_tensor(sabs[:], sims[:], sneg[:], op=ALU.max)
    rat = gp.tile([128, P], F32, tag="rat")
    nc.vector.tensor_tensor(rat[:], sims[:], rn[:], op=ALU.mult)
    u = gp.tile([128, P], F32, tag="u")
    nc.vector.tensor_tensor(u[:], rat[:], sabs[:], op=ALU.mult)

    mx = gp.tile([128, 8], F32, tag="mx")
    mi = gp.tile([128, 8], mybir.dt.uint32, tag="mi")
    nc.vector.max_with_indices(out_max=mx[:], out_indices=mi[:], in_=u[:])
    mif = gp.tile([128, 1], F32, tag="mif")
    nc.vector.tensor_copy(mif[:], mi[:, 0:1])
    gf = gp.tile([128, 1], F32, tag="gf")
    nc.vector.tensor_tensor(gf[:], mif[:], rowbase[:], op=ALU.add)
    gidx = gp.tile([128, 1], I32, tag="gidx")
    nc.vector.tensor_copy(gidx[:], gf[:])

    vb = gp.tile([128, DD], F32, tag="vb")
    nc.gpsimd.indirect_dma_start(
        out=vb[:],
        out_offset=None,
        in_=v_flat,
        in_offset=bass.IndirectOffsetOnAxis(ap=gidx[:], axis=0),
    )
    vbs = gp.tile([128, DD], F32, tag="vbs")
    nb2 = gp.tile([128, 1], F32, tag="nb2")
    nc.scalar.activation(out=vbs[:], in_=vb[:], func=AF.Square, accum_out=nb2[:])
    nbr = gp.tile([128, 1], F32, tag="nbr")
    nc.vector.reciprocal(nbr[:], nb2[:])
    nbs = gp.tile([128, 1], F32, tag="nbs")
    nc.scalar.activation(out=nbs[:], in_=nbr[:], func=AF.Sqrt)
    vbn = gp.tile([128, DD], F32, tag="vbn")
    nc.scalar.activation(out=vbn[:], in_=vb[:], func=AF.Copy, scale=nbs[:])

    # vbT[kc] = [128 k, 128 my-images] in bf16 to match the gathered t
    vbT = [gp.tile([128, 128], BF16, tag=f"vbT{k}", name=f"vbT{k}") for k in range(8)]
    for kc in range(8):
        pt = psum_tp.tile([128, 128], F32, tag="tp")
        nc.tensor.transpose(
            out=pt[:], in_=vbn[:, kc * 128 : (kc + 1) * 128], identity=ident[:]
        )
        nc.vector.tensor_copy(vbT[kc][:], pt[:])

    # out rows chunk r (rank r's images) = agT(r).T @ vbT
    o_sb = gp.tile([128, 8, 128], F32, tag="o_sb")
    for r in range(8):
        po = psum_o.tile([128, 128], F32, tag="po")
        for kc in range(8):
            nc.tensor.matmul(
                out=po[:],
                lhsT=agT[:, r, kc * 128 : (kc + 1) * 128],
                rhs=vbT[kc][:],
                start=(kc == 0),
                stop=(kc == 7),
            )
        nc.scalar.activation(out=o_sb[:, r, :], in_=po[:], func=AF.Copy)
        nc.sync.dma_start(out=o_d[r * 128 : (r + 1) * 128, :], in_=o_sb[:, r, :])

    ctx.close()


_CACHE = {}


def build():
    if "nc" in _CACHE:
        return _CACHE["nc"]
    nc = bacc.Bacc(
        "TRN2", target_bir_lowering=False, debug=False, num_devices=NCORES
    )
    v_d = nc.dram_tensor("v", [BS, P, DD], F32, kind="ExternalInput").ap()
    x_d = nc.dram_tensor("x", [BS, DC], F32, kind="ExternalInput").ap()
    w_d = nc.dram_tensor("w", [DD, DC], F32, kind="ExternalInput").ap()
    b_d = nc.dram_tensor("bv", [1, DD], F32, kind="ExternalInput").ap()
    o_d = nc.dram_tensor("out", [B, BS], F32, kind="ExternalOutput").ap()
    with tile.TileContext(nc) as tc:
        _build_kernel(tc, v_d, x_d, w_d, b_d, o_d)
    nc.compile()
    _CACHE["nc"] = nc
    return nc


def make_in_maps(visual_embedding, textual_embedding, W, b):
    in_maps = []
    for c in range(NCORES):
        sl = slice(c * BS, (c + 1) * BS)
        in_maps.append(
            {
                "v": np.ascontiguousarray(visual_embedding[sl], dtype=np.float32),
                "x": np.ascontiguousarray(textual_embedding[sl], dtype=np.float32),
                "w": np.ascontiguousarray(W, dtype=np.float32),
                "bv": np.ascontiguousarray(b, dtype=np.float32).reshape(1, DD),
            }
        )
    return in_maps


def kernel(visual_embedding, textual_embedding, W, b, _trace=False):
    nc = build()
    in_maps = make_in_maps(visual_embedding, textual_embedding, W, b)
    res = run_bass_kernel_spmd(nc, in_maps, list(range(NCORES)), trace=_trace)
    out = np.concatenate([res.results[c]["out"] for c in range(NCORES)], axis=1)
    if _trace:
        kernel.last_exec_time_ns = res.exec_time_ns
        kernel.last_profile = res.profile_json
        kernel.last_trace = (
            res.instructions_and_trace[1] if res.instructions_and_trace else None
        )
    return out
